# revision 1
# baseline (speedup 1.0000x reference)
# kernel.py — Trainium2 Bass kernel for nn_Net_17188459119113 (quantized CNN).
#
# Pipeline (per reference.py):
#   xq = quant4(x); wq = quant4(conv_w)
#   y  = conv2d(xq, wq, VALID) + b; relu; maxpool 4x4/4; flatten
#   fq = quant4(flat); out = fq @ quant4(fc_w).T + fc_b
#
# Strategy: pure data-parallel over 8 NeuronCores (batch 8192 -> 1024/core).
# On device, everything runs in the integer domain (quantized values are
# small exact integers in fp16/fp32), with affine scales applied late:
#   - x is quantized on device via the fp32 round-to-nearest-even magic
#     constant trick (v + 1.5*2^23 - 1.5*2^23), scale folded as multiply.
#   - conv = banded matmul: K = (dj in 0..2) x (h in 0..27) = 84 partitions,
#     stationary = banded integer weights [84, 128] per M-chunk
#     (M = 16 oc x 8 i-rows), moving = 3 shifted copies of the image rows,
#     N = (16 batch x 24 j) = 384 columns per matmul.
#   - W-direction maxpool fused into the PSUM drain (vector reduce_max over
#     j-windows, free dim), output as fp16 integers (exact, |y| <= 441).
#   - H-direction pool after a 16-bit DMA xbar transpose, again reduce_max.
#   - bias+relu deferred past the (monotone) max pools; relu folded into
#     the FC quantization clamp.
#   - global flat-max via gpsimd partition reduce + AllReduce(max) across
#     the 8 cores; FC = 5 accumulating [128,10]x[128,128] fp16 matmuls.
# Output returned as [10, 1024] per core, transposed/concatenated on host.

import numpy as np

P = 128
B_CORE = 1024  # images per core
NB = 8  # b-blocks of 128 images
NCORES = 8
MAGIC = float(np.float32(12582912.0))  # 1.5 * 2**23: fp32 RNE rounding trick

_NC = None  # cached compiled Bass module (input-independent)


def _f32(v):
    return np.float32(v)


def _host_quant_scale(t):
    # mirrors reference _quant scale computation in fp32 arithmetic
    n = _f32(7.0)
    m = np.max(np.abs(t.astype(np.float32))).astype(np.float32)
    return _f32(_f32(m / n) + _f32(1e-8))


def _build_nc():
    import concourse.bass as bass
    import concourse.mybir as mybir
    from concourse import bacc, bass_isa
    from concourse.tile import TileContext

    f32 = mybir.dt.float32
    f16 = mybir.dt.float16
    AF = mybir.ActivationFunctionType
    OP = mybir.AluOpType

    # Bacc (not bare Bass): its compile() runs generate_event_semaphores /
    # move_matmul_waits_to_ldweights, which legalize multi-semaphore waits
    # down to the 1-wait-per-instruction hardware limit.
    nc = bacc.Bacc(None, num_devices=NCORES)

    x_in = nc.declare_dram_parameter("x", [P, 6272], f32, isOutput=False)
    w3_in = nc.declare_dram_parameter("w3", [84, 384], f16, isOutput=False)
    fw_in = nc.declare_dram_parameter("fw", [P, 50], f16, isOutput=False)
    cbf_in = nc.declare_dram_parameter("cbf", [P, 640], f32, isOutput=False)
    fb_in = nc.declare_dram_parameter("fb", [P, 1], f32, isOutput=False)
    scal_in = nc.declare_dram_parameter("scal", [P, 4], f32, isOutput=False)
    out_ext = nc.declare_dram_parameter("out", [10, B_CORE], f32, isOutput=True)

    xq_dram = nc.dram_tensor("xq_scratch", [B_CORE, 28, 28], f16)
    cc_in = nc.dram_tensor("cc_in", [1, 512], f32)
    cc_out = nc.dram_tensor("cc_out", [1, 512], f32, addr_space="Shared")

    with TileContext(nc, num_cores=NCORES) as tc:
        with tc.tile_pool(name="const", bufs=1) as cpool:
            w3sb = cpool.tile([84, 384], f16)
            fwsb = cpool.tile([P, 50], f16)
            cbf = cpool.tile([P, 640], f32)
            fb = cpool.tile([P, 1], f32)
            scal = cpool.tile([P, 4], f32)
            lmax = cpool.tile([P, 1], f32)
            magic = cpool.tile([P, 1], f32)
            # DVE-produced copies of small constants: consumers then need at
            # most one DVE semaphore + one DMA semaphore (walrus caps the
            # number of sync-wait commands per instruction).
            scal_a = cpool.tile([P, 4], f32)
            fb2 = cpool.tile([P, 1], f32)
            nc.vector.memset(magic[:, :], MAGIC)
            nc.sync.dma_start(out=w3sb[:, :], in_=w3_in[:, :])
            nc.sync.dma_start(out=fwsb[:, :], in_=fw_in[:, :])
            nc.sync.dma_start(out=cbf[:, :], in_=cbf_in[:, :])
            nc.sync.dma_start(out=fb[:, :], in_=fb_in[:, :])
            nc.sync.dma_start(out=scal[:, :], in_=scal_in[:, :])
            nc.vector.memset(lmax[:, :], -3.0e38)
            nc.vector.tensor_copy(out=scal_a[:, :], in_=scal[:, :])
            nc.vector.tensor_copy(out=fb2[:, :], in_=fb[:, :])

            flatr = []  # persistent per-b-block real (pre-relu) flat tiles
            with (
                tc.tile_pool(name="xq", bufs=1) as xqpool,
                tc.tile_pool(name="flatr", bufs=NB) as frpool,
                tc.tile_pool(name="small", bufs=2) as smpool,
            ):
              # ---------- Phase 1: quantize x to integers (fp16) ----------
              if True:
                xf = xqpool.tile([P, 6272], f32)
                t1 = xqpool.tile([P, 6272], f32)
                xq16 = xqpool.tile([P, 6272], f16)
                nc.sync.dma_start(out=xf[:, :], in_=x_in[:, :])
                nchunk = 4
                w = 6272 // nchunk
                for q in range(nchunk):
                    sl = slice(q * w, (q + 1) * w)
                    # t1 = x * (1/s_x) + MAGIC   (fma on ACT, RNE at int grid)
                    nc.scalar.activation(
                        out=t1[:, sl], in_=xf[:, sl], func=AF.Identity,
                        bias=magic[:, 0:1], scale=scal_a[:, 0:1],
                    )
                    # xq = t1 - MAGIC  -> round-to-nearest-even integers
                    nc.vector.tensor_scalar(
                        out=xq16[:, sl], in0=t1[:, sl],
                        scalar1=MAGIC, scalar2=None, op0=OP.subtract,
                    )
                xq_flat = xq_dram[:, :, :].rearrange("b h w -> (b h w)")
                nc.sync.dma_start(out=xq_flat, in_=xq16[:, :])
              with (
                tc.tile_pool(name="x3", bufs=2) as x3pool,
                tc.tile_pool(name="ps", bufs=2, space="PSUM") as pspool,
                tc.tile_pool(name="yph", bufs=2) as yphpool,
                tc.tile_pool(name="tr6", bufs=2) as trpool,
                tc.tile_pool(name="flati", bufs=2) as fipool,
              ):
                for bb in range(NB):
                    # ---------- Phase 2: conv (banded matmul) ----------
                    x3 = x3pool.tile([84, 3584], f16)
                    xv = xq_dram[bb * P:(bb + 1) * P, :, :]
                    for dj in range(3):
                        src = xv[:, :, dj:dj + 26].rearrange("b h w -> h b w")
                        dst = x3[28 * dj:28 * (dj + 1), :].rearrange(
                            "h (b w) -> h b w", w=28)[:, :, 0:26]
                        nc.sync.dma_start(out=dst, in_=src)

                    flati = fipool.tile([P, 640], f16)
                    nc.vector.memset(flati[:, 576:640], 0.0)
                    fr = frpool.tile([P, 640], f32)
                    flatr.append(fr)

                    x3v = x3[:, :].rearrange("p (b w) -> p b w", w=28)
                    # wait-ladder: one tiny PE matmul per x3 dj-block so the
                    # real matmuls never need >1 DMA semaphore wait (walrus
                    # caps sync-wait commands per compute instruction at 2).
                    ps0 = pspool.tile([P, 2048], f32, tag="ps")
                    for pbase in (0, 32, 64):  # in dj-blocks 0/1/2 resp.
                        nc.tensor.matmul(
                            out=ps0[0:1, 0:1],
                            lhsT=x3[pbase:pbase + 1, 0:1],
                            rhs=x3[pbase:pbase + 1, 0:1],
                            start=True, stop=True,
                        )
                    for c in range(3):
                        yph = yphpool.tile([P, 768], f16)
                        for bsq in range(2):
                            ps = pspool.tile([P, 2048], f32, tag="ps")
                            for g in range(4):
                                bs = bsq * 4 + g
                                rhs = x3v[:, bs * 16:(bs + 1) * 16, 0:24]
                                nc.tensor.matmul(
                                    out=ps[:, g * 512:g * 512 + 384],
                                    lhsT=w3sb[:, c * 128:(c + 1) * 128],
                                    rhs=rhs, start=True, stop=True,
                                )
                            # W-pool: reduce max over j-windows of 4
                            pin = ps[:, :].rearrange(
                                "p (g s) -> p g s", g=4)[:, :, 0:384].rearrange(
                                "p g (b jj u) -> p g b jj u", b=16, jj=6, u=4)
                            yout = yph[:, :].rearrange(
                                "p (jj bq g b) -> p bq g b jj",
                                jj=6, bq=2, g=4, b=16)[:, bsq]
                            nc.vector.tensor_reduce(
                                out=yout, in_=pin,
                                axis=mybir.AxisListType.X, op=OP.max,
                            )
                        # ---------- Phase 3: transpose + H-pool ----------
                        for jj in range(6):
                            tr = trpool.tile([P, 128], f16)
                            nc.scalar.dma_start(
                                out=tr[:, :],
                                in_=yph[:, jj * 128:(jj + 1) * 128],
                                transpose=True,
                            )
                            tin = tr[:, :].rearrange(
                                "p (oc t u) -> p oc t u", oc=16, t=2, u=4)
                            tout = flati[:, 0:576].rearrange(
                                "p (oc ii jj) -> p oc ii jj", oc=16, ii=6, jj=6
                            )[:, :, 2 * c:2 * c + 2, jj]
                            nc.vector.tensor_reduce(
                                out=tout, in_=tin,
                                axis=mybir.AxisListType.X, op=OP.max,
                            )

                    # real pre-relu flat values: fr = s_xw * flati + conv_bias
                    nc.vector.tensor_scalar(
                        out=fr[:, :], in0=flati[:, :],
                        scalar1=scal_a[:, 1:2], scalar2=None, op0=OP.mult,
                    )
                    nc.vector.tensor_tensor(fr[:, :], fr[:, :], cbf[:, :], OP.add)
                    # local running max (pre-relu; relu applied to the max later)
                    tmp = smpool.tile([P, 1], f32)
                    nc.vector.tensor_reduce(
                        out=tmp[:, :], in_=fr[:, 0:576],
                        axis=mybir.AxisListType.X, op=OP.max,
                    )
                    nc.vector.tensor_tensor(lmax[:, :], lmax[:, :], tmp[:, :], OP.max)

              # ---------- Phase 4: global scale via AllReduce(max) ----------
              lmr = smpool.tile([P, 1], f32, tag="lmr")
              nc.gpsimd.partition_all_reduce(
                  lmr[:, :], lmax[:, :], 128, bass_isa.ReduceOp.max)
              bc = smpool.tile([1, 512], f32, tag="bc")
              nc.vector.tensor_copy(
                  out=bc[:, :], in_=lmr[0:1, 0:1].to_broadcast((1, 512)))
              nc.gpsimd.dma_start(out=cc_in[:, :], in_=bc[:, :])
              nc.gpsimd.collective_compute(
                  "AllReduce", OP.max,
                  replica_groups=[list(range(NCORES))],
                  ins=[cc_in[:, :]], outs=[cc_out[:, :]],
              )
              gm = smpool.tile([1, 1], f32, tag="gm")
              nc.gpsimd.dma_start(out=gm[:, :], in_=cc_out[0:1, 0:1])
              gmb = smpool.tile([P, 1], f32, tag="gmb")
              nc.gpsimd.partition_broadcast(gmb[:, :], gm[:, :], channels=P)
              # s_f = relu(gmax)/7 + 1e-8 ; invsf = 1/s_f ; sprod = s_f*s_fw
              rg = smpool.tile([P, 1], f32, tag="rg")
              nc.scalar.activation(out=rg[:, :], in_=gmb[:, :], func=AF.Relu)
              sf = smpool.tile([P, 1], f32, tag="sf")
              nc.vector.tensor_scalar(
                  out=sf[:, :], in0=rg[:, :],
                  scalar1=float(np.float32(1.0) / np.float32(7.0)),
                  scalar2=float(np.float32(1e-8)),
                  op0=OP.mult, op1=OP.add,
              )
              invsf = smpool.tile([P, 1], f32, tag="invsf")
              nc.vector.reciprocal(out=invsf[:, :], in_=sf[:, :])
              zerot = smpool.tile([P, 1], f32, tag="zerot")
              nc.vector.memset(zerot[:, :], 0.0)
              sprod = smpool.tile([P, 1], f32, tag="sprod")
              nc.vector.tensor_scalar(
                  out=sprod[:, :], in0=sf[:, :],
                  scalar1=scal_a[:, 2:3], scalar2=None, op0=OP.mult,
              )

              # ---------- Phase 5: FC ----------
              with (
                  tc.tile_pool(name="fq", bufs=2) as fqpool,
                  tc.tile_pool(name="fqt", bufs=3) as fqtpool,
                  tc.tile_pool(name="psfc", bufs=2, space="PSUM") as pfcpool,
                  tc.tile_pool(name="outp", bufs=2) as outpool,
              ):
                  for bb in range(NB):
                      fr = flatr[bb]
                      qt = fqpool.tile([P, 640], f32, tag="qt")
                      # relu + scale to quant grid: qt = max(fr*invsf, 0)
                      # (invsf > 0, so this equals max(fr,0)*invsf)
                      nc.vector.tensor_scalar(
                          out=qt[:, :], in0=fr[:, :],
                          scalar1=invsf[:, 0:1], scalar2=zerot[:, 0:1],
                          op0=OP.mult, op1=OP.max,
                      )
                      # round to nearest-even integers via magic add/sub
                      nc.scalar.activation(
                          out=qt[:, :], in_=qt[:, :], func=AF.Identity,
                          bias=magic[:, 0:1], scale=1.0,
                      )
                      fq = fqpool.tile([P, 640], f16, tag="fq")
                      nc.vector.tensor_scalar(
                          out=fq[:, :], in0=qt[:, :],
                          scalar1=MAGIC, scalar2=None, op0=OP.subtract,
                      )
                      psfc = pfcpool.tile([10, 128], f32)
                      for ks in range(5):
                          fqt = fqtpool.tile([P, 128], f16)
                          nc.scalar.dma_start(
                              out=fqt[:, :],
                              in_=fq[:, ks * 128:(ks + 1) * 128],
                              transpose=True,
                          )
                          nc.tensor.matmul(
                              out=psfc[:, :],
                              lhsT=fwsb[:, ks * 10:(ks + 1) * 10],
                              rhs=fqt[:, :],
                              start=(ks == 0), stop=(ks == 4),
                          )
                      osb = outpool.tile([10, 128], f32)
                      nc.scalar.activation(
                          out=osb[:, :], in_=psfc[:, :], func=AF.Identity,
                          bias=fb2[0:10, 0:1], scale=sprod[0:10, 0:1],
                      )
                      nc.sync.dma_start(
                          out=out_ext[:, bb * 128:(bb + 1) * 128], in_=osb[:, :])

    # Bacc passes: legalize multi-sem waits, fuse nops, codegen ISA subclasses.
    nc.finalize()
    return nc


def _host_constants(x, conv_w, conv_b, fc_w, fc_b):
    s_x = _host_quant_scale(x)
    s_w = _host_quant_scale(conv_w)
    s_fw = _host_quant_scale(fc_w)
    kw = np.round(conv_w.astype(np.float32) / s_w).astype(np.float32)
    kfw = np.round(fc_w.astype(np.float32) / s_fw).astype(np.float32)

    # banded conv weight matrix: W3[(dj,h), c*128 + oc*8 + isub] = kw[oc, h-i, dj]
    w3 = np.zeros((84, 384), np.float32)
    for dj in range(3):
        for c in range(3):
            for isub in range(8):
                i = 8 * c + isub
                for di in range(3):
                    h = i + di
                    if h < 28:
                        for oc in range(16):
                            w3[28 * dj + h, c * 128 + oc * 8 + isub] = kw[oc, 0, di, dj]

    # FC weights: fw[p, ks*10 + cls] = kfw[cls, ks*128 + p] (zero-padded)
    fw = np.zeros((P, 50), np.float32)
    for ks in range(5):
        for p in range(P):
            k = ks * 128 + p
            if k < 576:
                fw[p, ks * 10:(ks + 1) * 10] = kfw[:, k]

    # broadcast conv-bias pattern over flat index k = oc*36 + ii*6 + jj
    cbf_row = np.zeros((640,), np.float32)
    for k in range(576):
        cbf_row[k] = conv_b[k // 36]
    cbf = np.tile(cbf_row[None, :], (P, 1)).astype(np.float32)

    fb = np.zeros((P, 1), np.float32)
    fb[:10, 0] = fc_b.astype(np.float32)

    inv_sx = _f32(_f32(1.0) / s_x)
    s_xw = _f32(s_x * s_w)
    scal = np.tile(
        np.array([inv_sx, s_xw, s_fw, 0.0], np.float32)[None, :], (P, 1))

    return {
        "w3": w3.astype(np.float16),
        "fw": fw.astype(np.float16),
        "cbf": cbf,
        "fb": fb,
        "scal": scal.astype(np.float32),
    }


def _get_nc():
    global _NC
    if _NC is None:
        _NC = _build_nc()
    return _NC


def kernel(x, conv_w, conv_b, fc_w, fc_b, _trace=False):
    from concourse.bass_utils import run_bass_kernel_spmd

    x = np.asarray(x, np.float32)
    consts = _host_constants(
        x, np.asarray(conv_w, np.float32), np.asarray(conv_b, np.float32),
        np.asarray(fc_w, np.float32), np.asarray(fc_b, np.float32))

    nc = _get_nc()
    in_maps = []
    for c in range(NCORES):
        shard = x[c * B_CORE:(c + 1) * B_CORE].reshape(P, 6272)
        m = {"x": np.ascontiguousarray(shard)}
        m.update(consts)
        in_maps.append(m)

    res = run_bass_kernel_spmd(nc, in_maps, list(range(NCORES)), trace=_trace)
    out = np.concatenate([r["out"].T for r in res.results], axis=0)
    if _trace:
        kernel._last_results = res
    return np.ascontiguousarray(out.astype(np.float32))

